# revision 4
# baseline (speedup 1.0000x reference)
"""Trainium2 Bass kernel v2 for nn_AttLayer (sparse sliding-window attention).

Key changes vs baseline:
- Z (softmax denominator) computed via a DVE/Pool pairwise-combine tree over
  the 8 exp'd window chunks + ONE 512-cycle ones-matmul (baseline: 8
  accumulating ones-matmuls = 4096 PE cycles/block).
- Window/edge masking folded into the exp activation as per-partition bias
  (-40 => exp ~ 0), removing the DVE mask multiplies.
- Software-pipelined emission: per iter i the PE runs
  [E(i) g0,g1 | q-proj(i+3) | AV(i-1) c0-3 | E(i) g2 | AV(i-1) c4-7 |
   O(i-2) | E(i) g3 | k,v-proj(i+3) | Z(i-1)], with exp(i) on ScalarE,
  combine(i)+evacs on DVE/Pool, so AV never waits on same-block exps.
- PSUM packed into exactly 8 banks: et-ring [128,1024]x2 (E tiles + O output
  share one ring), mm-ring [128,512]x2 (q/k/v proj + Z), u-ring [128,512]x2.
- Engine balance per block (ns): PE 5333 | ScalarE ~5300 | DVE ~5000 |
  Pool ~3400.
"""

import math
import os
from contextlib import ExitStack

import numpy as np
import ml_dtypes

import concourse.bass as bass
import concourse.mybir as mybir
import concourse.tile as tile
from concourse import bacc

L = 131072
C = 256          # x1 / output channels
CH = 128         # q/k/v channels
NCORES = 8
BL = 512
HALF = 256
WIN = 1024
S = L // NCORES          # 16384 output cols per core
NB = S // BL             # 32 blocks per core
NSTEP = NB + 1           # proj steps covering the extended shard
SCALE = 1.0 / math.sqrt(CH)
NEG = -40.0              # exp bias for masked positions

F32 = mybir.dt.float32
BF16 = mybir.dt.bfloat16

LAST_RESULTS = None  # BassKernelResults of the most recent run (for test.py)


def build_bass(nb=NB, v_bias=False, qk_bias=False):
    nstep = nb + 1
    ext = nstep * BL
    s_loc = nb * BL

    nc = bacc.Bacc()
    x_h = nc.dram_tensor("x", (C, ext), BF16, kind="ExternalInput")
    wq_h = nc.dram_tensor("wq", (2, CH, CH), BF16, kind="ExternalInput")
    wk_h = nc.dram_tensor("wk", (2, CH, CH), BF16, kind="ExternalInput")
    wv_h = nc.dram_tensor("wv", (2, CH, CH), BF16, kind="ExternalInput")
    wo_h = nc.dram_tensor("wo", (2, CH, CH), BF16, kind="ExternalInput")
    bq_h = nc.dram_tensor("bq", (CH, 1), F32, kind="ExternalInput")
    bk_h = nc.dram_tensor("bk", (CH, 1), F32, kind="ExternalInput")
    bv_h = nc.dram_tensor("bv", (CH, 1), F32, kind="ExternalInput")
    # exp biases: b01 [128,2] masks window chunks 0,1 of block 0 (left halo);
    # b67 [128,2,nb] masks chunks 6,7 per block (window col 1023 always, plus
    # the right-halo padding of the last global block).
    b01_h = nc.dram_tensor("b01", (CH, 2), F32, kind="ExternalInput")
    b67_h = nc.dram_tensor("b67", (CH, 2, nb), F32, kind="ExternalInput")
    out_h = nc.dram_tensor("out", (C, s_loc), BF16, kind="ExternalOutput")

    x_r = x_h[:].rearrange("(g p) l -> p g l", p=CH)
    out_r = out_h[:].rearrange("(m p) l -> p m l", p=CH)

    EXP = mybir.ActivationFunctionType.Exp

    with tile.TileContext(nc) as tc, ExitStack() as ctx:
        singles = ctx.enter_context(tc.tile_pool(name="singles", bufs=1))
        xpool = ctx.enter_context(tc.tile_pool(name="xpool", bufs=3))
        ppool = ctx.enter_context(tc.tile_pool(name="ppool", bufs=2))
        cpool = ctx.enter_context(tc.tile_pool(name="cpool", bufs=2))
        rpool = ctx.enter_context(tc.tile_pool(name="rpool", bufs=2))
        opool = ctx.enter_context(tc.tile_pool(name="opool", bufs=2))
        ps_et = ctx.enter_context(tc.tile_pool(name="ps_et", bufs=2, space="PSUM"))
        ps_mm = ctx.enter_context(tc.tile_pool(name="ps_mm", bufs=2, space="PSUM"))
        ps_u = ctx.enter_context(tc.tile_pool(name="ps_u", bufs=2, space="PSUM"))

        q_all = singles.tile([CH, ext], BF16)
        k_all = singles.tile([CH, ext], BF16)
        vT_all = singles.tile([CH, ext], BF16)

        wq_sb = singles.tile([CH, 2, CH], BF16)
        wk_sb = singles.tile([CH, 2, CH], BF16)
        wv_sb = singles.tile([CH, 2, CH], BF16)
        wo_sb = singles.tile([CH, 2, CH], BF16)
        nc.gpsimd.dma_start(out=wq_sb, in_=wq_h[:].rearrange("g p m -> p g m"))
        nc.gpsimd.dma_start(out=wk_sb, in_=wk_h[:].rearrange("g p m -> p g m"))
        nc.gpsimd.dma_start(out=wv_sb, in_=wv_h[:].rearrange("g p m -> p g m"))
        nc.gpsimd.dma_start(out=wo_sb, in_=wo_h[:].rearrange("g p m -> p g m"))

        bq_sb = singles.tile([CH, 1], F32)
        bk_sb = singles.tile([CH, 1], F32)
        bv_sb = singles.tile([CH, 1], F32)
        nc.gpsimd.dma_start(out=bq_sb, in_=bq_h[:])
        nc.gpsimd.dma_start(out=bk_sb, in_=bk_h[:])
        nc.gpsimd.dma_start(out=bv_sb, in_=bv_h[:])
        b01_sb = singles.tile([CH, 2], F32)
        b67_sb = singles.tile([CH, 2, nb], F32)
        nc.gpsimd.dma_start(out=b01_sb, in_=b01_h[:])
        nc.gpsimd.dma_start(out=b67_sb, in_=b67_h[:])

        ones_sb = singles.tile([CH, CH], BF16)
        nc.vector.memset(ones_sb, 1.0)

        # ---- per-iteration state -------------------------------------
        xt = {}          # step -> x tile in sbuf
        proj_ps = {}     # step -> dict(q=, k=, v=) psum slots
        p_sb = {}        # block -> [128, 8*BL] exp'd P
        s8 = {}          # block -> [128, BL] chunk-combined sum
        z_ps = {}        # block -> [128, BL] psum Z
        u_ps = {}        # block -> [128, BL] psum AV accumulator
        r_sb = {}        # block -> [128, BL] bf16 relu(u)/Z
        et = {}          # block -> list of 4 [128, 2*BL] psum E tiles
        o_slot = {}      # block -> [128, 2*BL] psum O tile (et ring)
        o_sb = {}        # block -> [128, 2, BL] bf16 staging

        def load_x(s):
            t = xpool.tile([CH, 2, BL], BF16, tag="xt", name="xt")
            nc.sync.dma_start(out=t, in_=x_r[:, :, s * BL:(s + 1) * BL])
            xt[s] = t

        def emit_q_mm(s):
            q_ps = ps_mm.tile([CH, BL], F32, tag="mm", name="q_ps")
            xs = xt[s]
            nc.tensor.matmul(q_ps, wq_sb[:, 0], xs[:, 0], start=True, stop=False)
            nc.tensor.matmul(q_ps, wq_sb[:, 1], xs[:, 1], start=False, stop=True)
            proj_ps.setdefault(s, {})["q"] = q_ps

        def emit_kv_mm(s):
            xs = xt[s]
            k_ps = ps_mm.tile([CH, BL], F32, tag="mm", name="k_ps")
            nc.tensor.matmul(k_ps, wk_sb[:, 0], xs[:, 0], start=True, stop=False)
            nc.tensor.matmul(k_ps, wk_sb[:, 1], xs[:, 1], start=False, stop=True)
            v_ps = ps_mm.tile([CH, BL], F32, tag="mm", name="v_ps")
            for s4 in range(4):
                ssl = slice(s4 * CH, (s4 + 1) * CH)
                nc.tensor.matmul(v_ps[:, ssl], xs[:, 0, ssl], wv_sb[:, 0],
                                 start=True, stop=False)
                nc.tensor.matmul(v_ps[:, ssl], xs[:, 1, ssl], wv_sb[:, 1],
                                 start=False, stop=True)
            proj_ps[s]["k"] = k_ps
            proj_ps[s]["v"] = v_ps

        def emit_q_ev(s):
            sl = slice(s * BL, (s + 1) * BL)
            if qk_bias:
                nc.vector.tensor_scalar_add(q_all[:, sl], proj_ps[s]["q"], bq_sb)
            else:
                nc.vector.tensor_copy(q_all[:, sl], proj_ps[s]["q"])

        def emit_kv_ev(s):
            sl = slice(s * BL, (s + 1) * BL)
            if qk_bias:
                nc.vector.tensor_scalar_add(k_all[:, sl], proj_ps[s]["k"], bk_sb)
            else:
                nc.vector.tensor_copy(k_all[:, sl], proj_ps[s]["k"])
            nc.vector.tensor_copy(vT_all[:, sl], proj_ps[s]["v"])
            del proj_ps[s], xt[s]

        def emit_E_g(bi, g):
            t = ps_et.tile([CH, 2 * BL], F32, tag="et", name="et")
            q_blk = q_all[:, HALF + bi * BL: HALF + (bi + 1) * BL]
            for h in range(2):
                wc = 2 * g + h
                nc.tensor.matmul(
                    t[:, h * BL:(h + 1) * BL],
                    k_all[:, bi * BL + wc * CH: bi * BL + (wc + 1) * CH],
                    q_blk, start=True, stop=True)
            et.setdefault(bi, [None] * 4)[g] = t

        def emit_exp_g(bi, g):
            if bi not in p_sb:
                p_sb[bi] = ppool.tile([CH, 8 * BL], BF16, tag="p", name="p_sb")
            p = p_sb[bi]
            t = et[bi][g]
            dst = p[:, g * 2 * BL:(g + 1) * 2 * BL]
            if g == 3:
                nc.scalar.activation(dst[:, :BL], t[:, :BL], func=EXP,
                                     bias=b67_sb[:, 0, bi:bi + 1])
                nc.scalar.activation(dst[:, BL:], t[:, BL:], func=EXP,
                                     bias=b67_sb[:, 1, bi:bi + 1])
            elif g == 0 and bi == 0:
                nc.scalar.activation(dst[:, :BL], t[:, :BL], func=EXP,
                                     bias=b01_sb[:, 0:1])
                nc.scalar.activation(dst[:, BL:], t[:, BL:], func=EXP,
                                     bias=b01_sb[:, 1:2])
            else:
                nc.scalar.activation(dst, t, func=EXP)

        def cslice(bi, wc):
            return p_sb[bi][:, wc * BL:(wc + 1) * BL]

        def emit_pool_combine(bi):
            # p1 = c0+c1, p2 = c2+c3, q1 = p1+p2   (GpSimd)
            p1 = cpool.tile([CH, BL], BF16, tag="p1", name="p1")
            p2 = cpool.tile([CH, BL], BF16, tag="p2", name="p2")
            q1 = cpool.tile([CH, BL], BF16, tag="q1", name="q1")
            nc.gpsimd.tensor_tensor(p1, cslice(bi, 0), cslice(bi, 1),
                                    mybir.AluOpType.add)
            nc.gpsimd.tensor_tensor(p2, cslice(bi, 2), cslice(bi, 3),
                                    mybir.AluOpType.add)
            nc.gpsimd.tensor_tensor(q1, p1, p2, mybir.AluOpType.add)
            return q1

        def emit_dve_combine(bi, q1):
            # p3 = c4+c5, p4 = c6+c7   (DVE); Z sums q1+p3+p4 on the PE
            p3 = cpool.tile([CH, BL], BF16, tag="p3", name="p3")
            p4 = cpool.tile([CH, BL], BF16, tag="p4", name="p4")
            nc.vector.tensor_tensor(p3, cslice(bi, 4), cslice(bi, 5),
                                    mybir.AluOpType.add)
            nc.vector.tensor_tensor(p4, cslice(bi, 6), cslice(bi, 7),
                                    mybir.AluOpType.add)
            s8[bi] = (q1, p3, p4)

        def emit_AV(bi, first_half):
            if first_half:
                u_ps[bi] = ps_u.tile([CH, BL], F32, tag="u", name="u_ps")
            u = u_ps[bi]
            rng = range(0, 4) if first_half else range(4, 8)
            for wc in rng:
                vt = vT_all[:, (bi + wc // 4) * BL + (wc % 4) * CH:
                            (bi + wc // 4) * BL + (wc % 4 + 1) * CH]
                nc.tensor.matmul(u, vt, cslice(bi, wc),
                                 start=(wc == 0), stop=(wc == 7))

        def emit_Z(bi):
            z = ps_mm.tile([CH, BL], F32, tag="mm", name="z_ps")
            parts = s8[bi]
            for j, part in enumerate(parts):
                nc.tensor.matmul(z, ones_sb, part,
                                 start=(j == 0), stop=(j == len(parts) - 1))
            z_ps[bi] = z
            del s8[bi]

        def emit_recip_mult(bi):
            rz = rpool.tile([CH, BL], F32, tag="rz", name="rz")
            nc.vector.reciprocal_approx_fast(rz, z_ps[bi])
            r = rpool.tile([CH, BL], BF16, tag="r", name="r_sb")
            if v_bias:
                t = rpool.tile([CH, BL], F32, tag="t", name="t_sb")
                nc.vector.tensor_tensor(t, u_ps[bi], rz, mybir.AluOpType.mult)
                nc.vector.tensor_scalar(
                    out=r, in0=t, scalar1=bv_sb, scalar2=0.0,
                    op0=mybir.AluOpType.add, op1=mybir.AluOpType.max)
            else:
                # r = relu(u) * (1/Z)   (valid since Z > 0 and bv == 0)
                nc.vector.scalar_tensor_tensor(
                    out=r, in0=u_ps[bi], scalar=0.0, in1=rz,
                    op0=mybir.AluOpType.max, op1=mybir.AluOpType.mult)
            r_sb[bi] = r
            del z_ps[bi], u_ps[bi], p_sb[bi]

        def emit_O(bi):
            t = ps_et.tile([CH, 2 * BL], F32, tag="et", name="o_ps")
            for m in range(2):
                nc.tensor.matmul(t[:, m * BL:(m + 1) * BL], wo_sb[:, m],
                                 r_sb[bi], start=True, stop=True)
            o_slot[bi] = t
            del r_sb[bi]

        def emit_o_ev_a(bi):
            t = opool.tile([CH, 2, BL], BF16, tag="o", name="o_sb")
            nc.scalar.copy(t[:, 0], o_slot[bi][:, :BL])
            o_sb[bi] = t

        def emit_o_ev_b(bi):
            nc.vector.tensor_copy(o_sb[bi][:, 1], o_slot[bi][:, BL:])
            del o_slot[bi]

        def emit_out_dma(bi):
            nc.sync.dma_start(out=out_r[:, :, bi * BL:(bi + 1) * BL],
                              in_=o_sb[bi])
            del o_sb[bi]

        # ---- prologue: x loads for steps 0..3, proj steps 0..2 -------
        for s in range(4):
            load_x(s)
        emit_q_mm(0)
        emit_kv_mm(0)
        emit_q_ev(0)
        emit_q_mm(1)
        emit_kv_ev(0)
        emit_kv_mm(1)
        emit_q_ev(1)
        emit_q_mm(2)
        emit_kv_ev(1)
        emit_kv_mm(2)
        emit_q_ev(2)
        emit_kv_ev(2)

        # ---- main software-pipelined loop ----------------------------
        for i in range(nb + 2):
            if i + 4 <= nstep - 1:
                load_x(i + 4)
            if i < nb:
                emit_E_g(i, 0)
                emit_exp_g(i, 0)
                emit_E_g(i, 1)
                emit_exp_g(i, 1)
            if i + 3 <= nstep - 1:
                emit_q_mm(i + 3)
            if 1 <= i <= nb:
                emit_AV(i - 1, True)
            if 2 <= i <= nb + 1:
                emit_recip_mult(i - 2)
            if i < nb:
                emit_E_g(i, 2)
                emit_exp_g(i, 2)
            if i + 3 <= nstep - 1:
                emit_q_ev(i + 3)
            q1 = None
            if i < nb:
                q1 = emit_pool_combine(i)
            if 2 <= i <= nb + 1:
                emit_O(i - 2)
                emit_o_ev_a(i - 2)
                emit_o_ev_b(i - 2)
                emit_out_dma(i - 2)
            if i < nb:
                emit_E_g(i, 3)
                emit_exp_g(i, 3)
            if i + 3 <= nstep - 1:
                emit_kv_mm(i + 3)
                emit_kv_ev(i + 3)
            if 1 <= i <= nb:
                emit_AV(i - 1, False)
            if i < nb:
                emit_dve_combine(i, q1)
            if 1 <= i <= nb:
                emit_Z(i - 1)

    nc.compile()
    return nc


_NC_CACHE = {}


def _get_nc(nb=NB, v_bias=False, qk_bias=False):
    key = (nb, v_bias, qk_bias)
    if key not in _NC_CACHE:
        _NC_CACHE[key] = build_bass(nb, v_bias, qk_bias)
    return _NC_CACHE[key]


def make_in_maps(x1, mask, Wq, bq, Wk, bk, Wv, bv, Wo, bo, nb=NB, ncores=NCORES):
    bf16 = ml_dtypes.bfloat16
    s_loc = nb * BL
    ext = s_loc + 2 * HALF

    x = np.asarray(x1, np.float32)[0]
    l_tot = x.shape[1]
    assert l_tot == s_loc * ncores, (x.shape, nb, ncores)

    wq_a = np.ascontiguousarray(
        (np.asarray(Wq, np.float32) * SCALE).T.reshape(2, CH, CH)).astype(bf16)
    wk_a = np.ascontiguousarray(
        np.asarray(Wk, np.float32).T.reshape(2, CH, CH)).astype(bf16)
    wv_a = np.ascontiguousarray(
        np.asarray(Wv, np.float32).T.reshape(2, CH, CH)).astype(bf16)
    woT = np.asarray(Wo, np.float32).T
    wo_a = np.ascontiguousarray(
        woT.reshape(CH, 2, CH).transpose(1, 0, 2)).astype(bf16)
    bq_a = (np.asarray(bq, np.float32) * SCALE).reshape(CH, 1)
    bk_a = np.asarray(bk, np.float32).reshape(CH, 1)
    bv_a = np.asarray(bv, np.float32).reshape(CH, 1)

    xp = np.zeros((C, l_tot + 2 * HALF), np.float32)
    xp[:, HALF:HALF + l_tot] = x
    xp = xp.astype(bf16)

    # validity of each padded position (global pad + user mask)
    pv = np.zeros(l_tot + 2 * HALF, np.float32)
    pv[HALF:HALF + l_tot] = np.asarray(mask, np.float32)[0, 0]

    in_maps = []
    for c in range(ncores):
        base = c * s_loc
        b01 = np.zeros((CH, 2), np.float32)
        for wc in range(2):
            b01[:, wc] = NEG * (1.0 - pv[base + wc * CH: base + (wc + 1) * CH])
        b67 = np.zeros((CH, 2, nb), np.float32)
        for bi in range(nb):
            for j, wc in enumerate((6, 7)):
                w0 = base + bi * BL + wc * CH
                b67[:, j, bi] = NEG * (1.0 - pv[w0: w0 + CH])
            b67[CH - 1, 1, bi] = NEG  # window col 1023 never attends
        in_maps.append({
            "x": np.ascontiguousarray(xp[:, base:base + ext]),
            "wq": wq_a, "wk": wk_a, "wv": wv_a, "wo": wo_a,
            "bq": bq_a, "bk": bk_a, "bv": bv_a,
            "b01": b01, "b67": b67,
        })
    return in_maps


def kernel(x1, mask, Wq, bq, Wk, bk, Wv, bv, Wo, bo):
    global LAST_RESULTS
    from concourse.bass_utils import run_bass_kernel_spmd

    v_bias = bool(np.any(np.asarray(bv, np.float32)))
    qk_bias = bool(np.any(np.asarray(bq, np.float32))
                   or np.any(np.asarray(bk, np.float32)))
    nc = _get_nc(NB, v_bias, qk_bias)
    in_maps = make_in_maps(x1, mask, Wq, bq, Wk, bk, Wv, bv, Wo, bo)
    res = run_bass_kernel_spmd(
        nc, in_maps, core_ids=list(range(NCORES)),
        trace=bool(os.environ.get("BASS_TRACE")),
    )
    LAST_RESULTS = res
    outs = [r["out"].astype(np.float32) for r in res.results]
    out = np.concatenate(outs, axis=1)[None]
    bo_a = np.asarray(bo, np.float32)
    if bo_a.any():
        out = out + bo_a[None, :, None]
    m = np.asarray(mask, np.float32)
    if not (m == 1.0).all():
        out = out * m[:, 0:1, :]
    return out.astype(np.float32)
